# revision 5
# baseline (speedup 1.0000x reference)
"""MoE gate (router) kernel for Trainium2, 8 NeuronCores.

Computes, for hidden_states [4, 4096, 2048] f32 and router weight [64, 2048] f32:
  logits = x @ w.T        -> softmax over 64 experts -> top-2 (+ normalized weights)
  plus the seq_aux load-balancing loss statistics.

Sharding: tokens (batch*seq = 16384) split contiguously across 8 cores (2048 each);
router weight replicated. Aux-loss statistics are reduced on host from tiny
per-core partial sums.
"""

import os
import numpy as np

import concourse.bass as bass
import concourse.mybir as mybir
import concourse.tile as tile
from concourse import bacc
from concourse.bass_utils import run_bass_kernel_spmd

F32 = mybir.dt.float32
I32 = mybir.dt.int32
U32 = mybir.dt.uint32

N_CORES = 8
BSZ = 4
SEQ = 4096
DIM = 2048
E = 64
TOP_K = 2
ALPHA = 0.01
N_TOKENS = BSZ * SEQ              # 16384
TOK_PER_CORE = N_TOKENS // N_CORES  # 2048
P = 128                           # partitions / tile height
N_TILES = TOK_PER_CORE // P       # 16 token tiles per core
H_CHUNKS = DIM // P               # 16 contraction chunks


def build_program():
    nc = bacc.Bacc("TRN2", target_bir_lowering=False)

    x_in = nc.dram_tensor("x", [TOK_PER_CORE, DIM], F32, kind="ExternalInput")
    # host-pretransposed router weight: wt[c, p, e] = w[e, c*128 + p]
    wt_in = nc.dram_tensor("wt", [H_CHUNKS, P, E], F32, kind="ExternalInput")
    ident_in = nc.dram_tensor("ident", [P, P], F32, kind="ExternalInput")

    # outputs stay in SBUF-native layout; host unscrambles
    out_w = nc.dram_tensor("out_w", [P, 2 * N_TILES], F32, kind="ExternalOutput")
    out_idx = nc.dram_tensor("out_idx", [P, 2 * N_TILES], I32, kind="ExternalOutput")
    # 2 groups x [8, 512]; host takes diagonal 64-blocks
    out_ssum = nc.dram_tensor("out_ssum", [16, 512], F32, kind="ExternalOutput")

    with tile.TileContext(nc) as tc:
        with (
            tc.tile_pool(name="consts", bufs=1) as consts,
            tc.tile_pool(name="xload", bufs=3) as xload,
            tc.tile_pool(name="xtsb", bufs=3) as xtsb,
            tc.tile_pool(name="soft", bufs=4) as soft,
            tc.tile_pool(name="acc", bufs=1) as acc,
            tc.tile_pool(name="ptp", bufs=2, space="PSUM") as ptp,
            tc.tile_pool(name="plp", bufs=2, space="PSUM") as plp,
            tc.tile_pool(name="pst", bufs=2, space="PSUM") as pst,
        ):
            ident_sb = consts.tile([P, P], F32)
            nc.sync.dma_start(ident_sb, ident_in[:, :])
            wt_sb = consts.tile([P, H_CHUNKS, E], F32)
            nc.sync.dma_start(wt_sb, wt_in.rearrange("c p e -> p c e"))

            ex_all = acc.tile([P, N_TILES * E], F32)     # exp(logit - max)
            rs_all = acc.tile([P, N_TILES], F32)         # 1/sumexp per token
            w_all = acc.tile([P, 2 * N_TILES], F32)
            idx_all = acc.tile([P, 2 * N_TILES], I32)

            for t in range(N_TILES):
                x_t = xload.tile([P, DIM], F32, tag="x")
                nc.sync.dma_start(x_t, x_in[t * P:(t + 1) * P, :])

                pl = plp.tile([P, E], F32, tag="logits")
                # 4 groups of 4 chunks: transpose into one PSUM bank, one
                # batched copy to SBUF, then 4 matmuls
                for g4 in range(4):
                    pt = ptp.tile([P, 4 * P], F32, tag="xt_ps")
                    for j in range(4):
                        c = g4 * 4 + j
                        nc.tensor.transpose(
                            pt[:, j * P:(j + 1) * P],
                            x_t[:, c * P:(c + 1) * P],
                            ident_sb,
                        )
                    xt = xtsb.tile([P, 4 * P], F32, tag="xt_sb")
                    nc.scalar.copy(xt, pt)
                    for j in range(4):
                        c = g4 * 4 + j
                        nc.tensor.matmul(
                            pl,
                            lhsT=xt[:, j * P:(j + 1) * P],
                            rhs=wt_sb[:, c, :],
                            start=(c == 0),
                            stop=(c == H_CHUNKS - 1),
                            skip_group_check=True,
                        )

                # ---- softmax + top-2 ----
                negmax = soft.tile([P, 1], F32, tag="negmax")
                nc.vector.tensor_reduce(
                    negmax, pl, axis=mybir.AxisListType.X,
                    op=mybir.AluOpType.max, negate=True,
                )
                ex_t = ex_all[:, t * E:(t + 1) * E]
                sumex = soft.tile([P, 1], F32, tag="sumex")
                nc.scalar.activation(
                    ex_t, pl, mybir.ActivationFunctionType.Exp,
                    bias=negmax, scale=1.0, accum_out=sumex,
                )
                nc.vector.reciprocal(rs_all[:, t:t + 1], sumex)

                m8 = soft.tile([P, 8], F32, tag="m8")
                i8 = soft.tile([P, 8], U32, tag="i8")
                nc.vector.max(out=m8, in_=ex_t)
                nc.vector.max_index(out=i8, in_max=m8, in_values=ex_t)

                s12 = soft.tile([P, 1], F32, tag="s12")
                nc.vector.tensor_reduce(
                    s12, m8[:, 0:2], axis=mybir.AxisListType.X,
                    op=mybir.AluOpType.add,
                )
                rs12 = soft.tile([P, 1], F32, tag="rs12")
                nc.vector.reciprocal(rs12, s12)
                nc.vector.tensor_scalar_mul(
                    w_all[:, 2 * t:2 * t + 2], m8[:, 0:2], rs12
                )
                nc.vector.tensor_copy(
                    idx_all[:, 2 * t:2 * t + 2], i8[:, 0:2].bitcast(I32)
                )

            # ---- aux-loss partial sums: sum_tok scores[tok, e] ----
            # diag-block trick: out[m, n] = sum_p rs_all[p, g*8+m] * ex_all[p, n]
            # valid entries are the m-th 64-block of row m.
            for g in range(2):
                ps = pst.tile([8, 512], F32, tag="stats")
                nc.tensor.matmul(
                    ps,
                    lhsT=rs_all[:, g * 8:(g + 1) * 8],
                    rhs=ex_all[:, g * 512:(g + 1) * 512],
                    start=True, stop=True,
                    skip_group_check=True,
                )
                ss = soft.tile([8, 512], F32, tag="ss")
                nc.scalar.copy(ss, ps)
                nc.sync.dma_start(out_ssum[g * 8:(g + 1) * 8, :], ss)

            nc.sync.dma_start(out_w[:, :], w_all[:, :])
            nc.sync.dma_start(out_idx[:, :], idx_all[:, :])

    if not nc.is_finalized():
        nc.finalize()
    return nc


_NC = None


def _get_nc():
    global _NC
    if _NC is None:
        _NC = build_program()
    return _NC


def _run(hidden_states, weight, trace=False, trace_kwargs=None):
    x = np.ascontiguousarray(
        np.asarray(hidden_states, dtype=np.float32).reshape(N_TOKENS, DIM)
    )
    w = np.asarray(weight, dtype=np.float32)
    wt = np.ascontiguousarray(w.T.reshape(H_CHUNKS, P, E))
    ident = np.eye(P, dtype=np.float32)

    shards = x.reshape(N_CORES, TOK_PER_CORE, DIM)
    in_maps = [
        {"x": np.ascontiguousarray(shards[c]), "wt": wt, "ident": ident}
        for c in range(N_CORES)
    ]
    nc = _get_nc()
    kw = {}
    if trace:
        kw["trace"] = True
        if trace_kwargs:
            kw["trace_kwargs"] = trace_kwargs
    out = run_bass_kernel_spmd(nc, in_maps, list(range(N_CORES)), **kw)
    return out


def _assemble(results):
    topk_w = np.empty((N_TOKENS, TOP_K), dtype=np.float32)
    topk_idx = np.empty((N_TOKENS, TOP_K), dtype=np.int32)
    ssum = np.empty((N_CORES, E), dtype=np.float32)
    for c, res in enumerate(results):
        wv = res["out_w"].reshape(P, N_TILES, TOP_K)      # [p, t, k]
        iv = res["out_idx"].reshape(P, N_TILES, TOP_K)
        base = c * TOK_PER_CORE
        topk_w[base:base + TOK_PER_CORE] = (
            wv.transpose(1, 0, 2).reshape(TOK_PER_CORE, TOP_K)
        )
        topk_idx[base:base + TOK_PER_CORE] = (
            iv.transpose(1, 0, 2).reshape(TOK_PER_CORE, TOP_K).astype(np.int32)
        )
        sm = res["out_ssum"].reshape(2, 8, 512)
        parts = np.empty((N_TILES, E), dtype=np.float32)
        for g in range(2):
            for m in range(8):
                parts[g * 8 + m] = sm[g, m, m * E:(m + 1) * E]
        ssum[c] = parts.sum(axis=0)

    # aux loss (host reduction, fp32)
    scores_seq_mean = ssum.reshape(BSZ, 2, E).sum(axis=1) / np.float32(SEQ)
    idx_b = topk_idx.reshape(BSZ, SEQ * TOP_K)
    ce = np.stack(
        [np.bincount(idx_b[b], minlength=E) for b in range(BSZ)]
    ).astype(np.float32)
    ce = ce / np.float32(SEQ * TOP_K / E)
    aux_loss = np.float32(
        (ce * scores_seq_mean).sum(axis=1).mean() * ALPHA
    )
    return topk_idx, topk_w, aux_loss


def kernel(hidden_states, weight):
    out = _run(hidden_states, weight)
    return _assemble(out.results)


# revision 7
# speedup vs baseline: 1.6516x; 1.6516x over previous
"""MoE gate (router) kernel for Trainium2, 8 NeuronCores.

Computes, for hidden_states [4, 4096, 2048] f32 and router weight [64, 2048] f32:
  logits = x @ w.T -> softmax over 64 experts -> top-2 (+ normalized weights)
  plus the seq_aux load-balancing loss.

Sharding: tokens (batch*seq = 16384) split contiguously across 8 cores
(2048 each); router weight replicated. Host pre-transposes each token shard
to [dim, tokens] layout and (in bf16x3 mode) splits fp32 into bf16 hi/lo
halves so the router GEMM runs at bf16 PE rate with ~fp32 accuracy
(error ~2^-17; products are exact, only psum accumulate rounds).
Aux-loss statistics are reduced on host from tiny per-core partials.
"""

import os
import numpy as np
import ml_dtypes

import concourse.bass as bass
import concourse.mybir as mybir
import concourse.tile as tile
from concourse import bacc
from concourse.bass_utils import run_bass_kernel_spmd

F32 = mybir.dt.float32
BF16 = mybir.dt.bfloat16
I32 = mybir.dt.int32
U32 = mybir.dt.uint32
NP_BF16 = ml_dtypes.bfloat16

N_CORES = 8
BSZ = 4
SEQ = 4096
DIM = 2048
E = 64
TOP_K = 2
ALPHA = 0.01
N_TOKENS = BSZ * SEQ                 # 16384
TOK_PER_CORE = N_TOKENS // N_CORES   # 2048
P = 128
N_TILES = TOK_PER_CORE // P          # 16 token tiles per core
H_CHUNKS = DIM // P                  # 16 contraction chunks
N_GROUPS = 4                         # 512-token matmul groups
GTOK = TOK_PER_CORE // N_GROUPS      # 512

MODE = os.environ.get("BASS_MOE_MODE", "bf16x3")


def _softmax_top2(nc, soft, pl, t, ex_all, rs_all, w_all, idx_all):
    """pl: PSUM [128, 64] logits for token tile t. Emits softmax+top2."""
    negmax = soft.tile([P, 1], F32, tag="negmax")
    nc.vector.tensor_reduce(
        negmax, pl, axis=mybir.AxisListType.X,
        op=mybir.AluOpType.max, negate=True,
    )
    ex_t = ex_all[:, t * E:(t + 1) * E]
    sumex = soft.tile([P, 1], F32, tag="sumex")
    nc.scalar.activation(
        ex_t, pl, mybir.ActivationFunctionType.Exp,
        bias=negmax, scale=1.0, accum_out=sumex,
    )
    nc.vector.reciprocal(rs_all[:, t:t + 1], sumex)

    m8 = soft.tile([P, 8], F32, tag="m8")
    i8 = soft.tile([P, 8], U32, tag="i8")
    nc.vector.max(out=m8, in_=ex_t)
    nc.vector.max_index(out=i8, in_max=m8, in_values=ex_t)

    s12 = soft.tile([P, 1], F32, tag="s12")
    nc.vector.tensor_reduce(
        s12, m8[:, 0:2], axis=mybir.AxisListType.X, op=mybir.AluOpType.add,
    )
    rs12 = soft.tile([P, 1], F32, tag="rs12")
    nc.vector.reciprocal(rs12, s12)
    nc.vector.tensor_scalar_mul(w_all[:, 2 * t:2 * t + 2], m8[:, 0:2], rs12)
    nc.vector.tensor_copy(idx_all[:, 2 * t:2 * t + 2], i8[:, 0:2].bitcast(I32))


def build_program(mode=MODE):
    nc = bacc.Bacc("TRN2", target_bir_lowering=False)

    if mode == "f32":
        x_ins = [nc.dram_tensor("xt", [DIM, TOK_PER_CORE], F32,
                                kind="ExternalInput")]
        wt_ins = [nc.dram_tensor("wt", [H_CHUNKS, P, E], F32,
                                 kind="ExternalInput")]
        xdt, wdt = F32, F32
        # (stationary_idx, moving_idx) term list
        terms = [(0, 0)]
    else:
        x_ins = [
            nc.dram_tensor("xh", [DIM, TOK_PER_CORE], BF16, kind="ExternalInput"),
            nc.dram_tensor("xl", [DIM, TOK_PER_CORE], BF16, kind="ExternalInput"),
        ]
        wt_ins = [
            nc.dram_tensor("wth", [H_CHUNKS, P, E], BF16, kind="ExternalInput"),
            nc.dram_tensor("wtl", [H_CHUNKS, P, E], BF16, kind="ExternalInput"),
        ]
        xdt, wdt = BF16, BF16
        # order for stationary reuse: wth with xh and xl, then wtl with xh
        terms = [(0, 0), (0, 1), (1, 0)]
        if mode == "bf16x4":
            terms.append((1, 1))

    ident_in = nc.dram_tensor("ident", [P, P], F32, kind="ExternalInput")

    out_w = nc.dram_tensor("out_w", [P, 2 * N_TILES], F32, kind="ExternalOutput")
    out_idx = nc.dram_tensor("out_idx", [P, 2 * N_TILES], I32, kind="ExternalOutput")
    out_ssum = nc.dram_tensor("out_ssum", [16, 512], F32, kind="ExternalOutput")

    with tile.TileContext(nc) as tc:
        with (
            tc.tile_pool(name="consts", bufs=1) as consts,
            tc.tile_pool(name="xload", bufs=4) as xload,
            tc.tile_pool(name="lgsb", bufs=2) as lgsb,
            tc.tile_pool(name="soft", bufs=4) as soft,
            tc.tile_pool(name="acc", bufs=1) as acc,
            tc.tile_pool(name="plg", bufs=1, space="PSUM") as plg,
            tc.tile_pool(name="plp", bufs=2, space="PSUM") as plp,
            tc.tile_pool(name="pst", bufs=2, space="PSUM") as pst,
        ):
            ident_sb = consts.tile([P, P], F32)
            nc.sync.dma_start(ident_sb, ident_in[:, :])
            wt_sbs = []
            for wi, wt_in in enumerate(wt_ins):
                wsb = consts.tile([P, H_CHUNKS, E], wdt, tag=f"wt{wi}")
                nc.sync.dma_start(wsb, wt_in.rearrange("c p e -> p c e"))
                wt_sbs.append(wsb)

            ex_all = acc.tile([P, N_TILES * E], F32)
            rs_all = acc.tile([P, N_TILES], F32)
            w_all = acc.tile([P, 2 * N_TILES], F32)
            idx_all = acc.tile([P, 2 * N_TILES], I32)

            # accumulators for logits^T, one PSUM bank per 512-token group
            psum_lgT = [
                plg.tile([E, GTOK], F32, tag=f"lgT{g}", name=f"lgT{g}")
                for g in range(N_GROUPS)
            ]

            # ---- router GEMM: logits^T[e, tok] += wT_c^T @ xT_c ----
            # load 2 h-chunks per DMA (bigger transfers)
            n_loads = H_CHUNKS // 2
            x_tiles = {}
            for ld in range(n_loads):
                for xi, x_in in enumerate(x_ins):
                    xt2 = xload.tile([P, 2, TOK_PER_CORE], xdt, tag=f"x{xi}")
                    nc.sync.dma_start(
                        xt2,
                        x_in[ld * 2 * P:(ld + 1) * 2 * P, :].rearrange(
                            "(j p) t -> p j t", p=P),
                    )
                    x_tiles[xi] = xt2
                for jj in range(2):
                    c = ld * 2 + jj
                    for ti, (wi, xi) in enumerate(terms):
                        first = (c == 0 and ti == 0)
                        last = (c == H_CHUNKS - 1 and ti == len(terms) - 1)
                        for g in range(N_GROUPS):
                            nc.tensor.matmul(
                                psum_lgT[g],
                                lhsT=wt_sbs[wi][:, c, :],
                                rhs=x_tiles[xi][:, jj, g * GTOK:(g + 1) * GTOK],
                                start=first, stop=last,
                                skip_group_check=True,
                            )

            # ---- per group: transpose logits back to [tok, E], softmax ----
            for g in range(N_GROUPS):
                lgT_sb = lgsb.tile([E, GTOK], F32, tag="lgT_sb")
                nc.scalar.copy(lgT_sb, psum_lgT[g])
                for j in range(N_GROUPS):
                    t = g * 4 + j
                    pl = plp.tile([P, E], F32, tag="logits")
                    nc.tensor.transpose(
                        pl, lgT_sb[:, j * P:(j + 1) * P], ident_sb[:E, :E],
                    )
                    _softmax_top2(nc, soft, pl, t, ex_all, rs_all, w_all, idx_all)

            # ---- aux-loss partial sums (diag-block trick) ----
            for g in range(2):
                ps = pst.tile([8, 512], F32, tag="stats")
                nc.tensor.matmul(
                    ps,
                    lhsT=rs_all[:, g * 8:(g + 1) * 8],
                    rhs=ex_all[:, g * 512:(g + 1) * 512],
                    start=True, stop=True,
                    skip_group_check=True,
                )
                ss = soft.tile([8, 512], F32, tag="ss")
                nc.scalar.copy(ss, ps)
                nc.sync.dma_start(out_ssum[g * 8:(g + 1) * 8, :], ss)

            nc.sync.dma_start(out_w[:, :], w_all[:, :])
            nc.sync.dma_start(out_idx[:, :], idx_all[:, :])

    if not nc.is_finalized():
        nc.finalize()
    return nc


_NC = {}


def _get_nc(mode=MODE):
    if mode not in _NC:
        _NC[mode] = build_program(mode)
    return _NC[mode]


def _prep_inputs(hidden_states, weight, mode=MODE):
    x = np.asarray(hidden_states, dtype=np.float32).reshape(N_TOKENS, DIM)
    w = np.asarray(weight, dtype=np.float32)
    ident = np.eye(P, dtype=np.float32)

    in_maps = []
    if mode == "f32":
        wt = np.ascontiguousarray(w.T.reshape(H_CHUNKS, P, E))
        for c in range(N_CORES):
            xt = np.ascontiguousarray(
                x[c * TOK_PER_CORE:(c + 1) * TOK_PER_CORE, :].T)
            in_maps.append({"xt": xt, "wt": wt, "ident": ident})
    else:
        wh = w.astype(NP_BF16)
        wl = (w - wh.astype(np.float32)).astype(NP_BF16)
        wth = np.ascontiguousarray(wh.T.reshape(H_CHUNKS, P, E))
        wtl = np.ascontiguousarray(wl.T.reshape(H_CHUNKS, P, E))
        for c in range(N_CORES):
            xs = x[c * TOK_PER_CORE:(c + 1) * TOK_PER_CORE, :]
            xh = xs.astype(NP_BF16)
            xl = (xs - xh.astype(np.float32)).astype(NP_BF16)
            in_maps.append({
                "xh": np.ascontiguousarray(xh.T),
                "xl": np.ascontiguousarray(xl.T),
                "wth": wth, "wtl": wtl, "ident": ident,
            })
    return in_maps


def _run(hidden_states, weight, trace=False, mode=MODE, tmpdir=None):
    in_maps = _prep_inputs(hidden_states, weight, mode)
    nc = _get_nc(mode)
    kw = {}
    if trace:
        kw["trace"] = True
        if tmpdir:
            kw["tmpdir"] = tmpdir
    return run_bass_kernel_spmd(nc, in_maps, list(range(N_CORES)), **kw)


def _assemble(results):
    topk_w = np.empty((N_TOKENS, TOP_K), dtype=np.float32)
    topk_idx = np.empty((N_TOKENS, TOP_K), dtype=np.int32)
    ssum = np.empty((N_CORES, E), dtype=np.float32)
    for c, res in enumerate(results):
        wv = res["out_w"].reshape(P, N_TILES, TOP_K)
        iv = res["out_idx"].reshape(P, N_TILES, TOP_K)
        base = c * TOK_PER_CORE
        topk_w[base:base + TOK_PER_CORE] = (
            wv.transpose(1, 0, 2).reshape(TOK_PER_CORE, TOP_K))
        topk_idx[base:base + TOK_PER_CORE] = (
            iv.transpose(1, 0, 2).reshape(TOK_PER_CORE, TOP_K).astype(np.int32))
        sm = res["out_ssum"].reshape(2, 8, 512)
        parts = np.empty((N_TILES, E), dtype=np.float32)
        for g in range(2):
            for m in range(8):
                parts[g * 8 + m] = sm[g, m, m * E:(m + 1) * E]
        ssum[c] = parts.sum(axis=0)

    scores_seq_mean = ssum.reshape(BSZ, 2, E).sum(axis=1) / np.float32(SEQ)
    idx_b = topk_idx.reshape(BSZ, SEQ * TOP_K)
    ce = np.stack(
        [np.bincount(idx_b[b], minlength=E) for b in range(BSZ)]
    ).astype(np.float32)
    ce = ce / np.float32(SEQ * TOP_K / E)
    aux_loss = np.float32((ce * scores_seq_mean).sum(axis=1).mean() * ALPHA)
    return topk_idx, topk_w, aux_loss


def kernel(hidden_states, weight):
    out = _run(hidden_states, weight)
    return _assemble(out.results)
